# revision 25
# baseline (speedup 1.0000x reference)
"""MultiHeadAttention Trainium2 kernel (8 NeuronCores).

Sharding: core c handles batch b = c // 2 and head-group hg = c % 2
(8 of 16 heads, 512 of 1024 model dims). Attention is embarrassingly
parallel over (b, hg); the output projection is computed per head-group
against the matching W_o columns, yielding partial outputs that the host
sums (plus b_o).

Device dataflow (per core), all in "transposed" layouts so no on-device
transposes are ever needed:
  qT = Wq_hg @ Xq^T      [dh=512, S]   (lhsT = Wq_hg^T, rhs = Xq^T)
  kT = Wk_hg @ Xk^T      [dh=512, S]
  v  = Xv @ Wv_hg^T      [S, dh=512]   (+ ones column per head for sums)
  scores_T[k, q]: per head-pair (m) and half (hl), keys on partitions;
    the two hl matmuls run CONCURRENTLY as 64-row PE tiles (row_grp h0 /
    h64, auto-derived from lhsT base partitions) and write one 2-bank
    PSUM tile [128, 2, QB] so a single fused ACT exp covers the pair.
    Diagonal chunks restrict columns to the causally-reachable range;
    the 128-col triangular block is zeroed after exp with one small
    Pool-engine multiply.
  probs -> PV: attn_T[d, q] + sums row accumulated in PSUM [65, QB],
    rhs column-restricted per chunk.
  normalize: per head-pair reciprocal_approx_fast directly on the PSUM
    sums row (partition 64), cast to bf16, then a K=1 ones-matmul
    broadcast to 64 partitions and a DVE multiply — no DMA round-trips,
    no batched-recip barrier (norm pipelines per m-group).
  out_partial = attn^T-matmul with Wo columns.

Schedule (emission order == engine-queue order):
  V proj | K0 Q0 | K1..K3 Q1..Q3 interleaved with qb0+qb1 attention
  (generators yield per key-chunk so exp/PV ride the projection window)
  | qb0 norm+outproj interleaved with the qb1 attention drain
  | for qb in 2..: attn(qb) m-groups interleaved with norm+outproj(qb-1)
  | norm+outproj(last).
"""

import os

import numpy as np

B, S_FULL, D = 4, 2048, 1024
H, DK = 16, 64
NH_G = 8          # heads per core
DH = NH_G * DK    # 512 dims per core
P = 128
KC = 128          # key chunk (PE contraction)
SCALE = 1.0 / np.sqrt(np.float32(DK))

_PROG_CACHE = {}


def _dims(S):
    QB = min(512, S)
    return {
        "S": S, "QB": QB, "N_QB": S // QB, "N_KC": S // KC,
        "R": QB // KC, "E_CH": D // P, "M_CH": DH // P, "O_N": D // 512,
    }


def _np_dt(use_bf16):
    if use_bf16:
        import ml_dtypes
        return ml_dtypes.bfloat16
    return np.float32


def build_program(causal, S, use_bf16=True):
    """Build the single-core Bass/Tile program (same program on all 8 cores)."""
    from contextlib import ExitStack

    import concourse.bass as bass
    import concourse.tile as tile
    from concourse import bacc, mybir

    d = _dims(S)
    QB, N_QB, N_KC, R, E_CH, M_CH, O_N = (
        d["QB"], d["N_QB"], d["N_KC"], d["R"], d["E_CH"], d["M_CH"], d["O_N"])

    DT = mybir.dt.bfloat16 if use_bf16 else mybir.dt.float32r
    F32 = mybir.dt.float32
    AF = mybir.ActivationFunctionType
    ALU = mybir.AluOpType

    WB = QB
    NW = S // WB

    nc = bacc.Bacc("TRN2", target_bir_lowering=False, debug=False)

    NB = S // QB
    xq_t = nc.dram_tensor("xq_t", [NB, P, E_CH, QB], DT,
                          kind="ExternalInput").ap()
    xk_t = nc.dram_tensor("xk_t", [NB, P, E_CH, QB], DT,
                          kind="ExternalInput").ap()
    xv_t = nc.dram_tensor("xv_t", [NB, P, E_CH, QB], DT,
                          kind="ExternalInput").ap()
    wq_t = nc.dram_tensor("wq_t", [P, E_CH, DH], DT,
                          kind="ExternalInput").ap()
    wk_t = nc.dram_tensor("wk_t", [P, E_CH, DH], DT,
                          kind="ExternalInput").ap()
    wv_t = nc.dram_tensor("wv_t", [P, E_CH, DH], DT,
                          kind="ExternalInput").ap()
    wo_t = nc.dram_tensor("wo_t", [P, M_CH, D], DT,
                          kind="ExternalInput").ap()
    bq_in = nc.dram_tensor("bq_p", [P, M_CH], F32, kind="ExternalInput").ap()
    bk_in = nc.dram_tensor("bk_p", [P, M_CH], F32, kind="ExternalInput").ap()
    bv_in = nc.dram_tensor("bv_r", [P, DH], F32, kind="ExternalInput").ap()
    dmask_in = nc.dram_tensor("dmask", [P, KC], DT,
                              kind="ExternalInput").ap()
    ones_v_in = nc.dram_tensor("ones_v", [P, N_KC, NH_G, 1], DT,
                               kind="ExternalInput").ap()
    out_p = nc.dram_tensor("out_p", [S, D], F32, kind="ExternalOutput").ap()

    with tile.TileContext(nc) as tc, ExitStack() as ctx:
        consts = ctx.enter_context(tc.tile_pool(name="consts", bufs=1))
        wpool = ctx.enter_context(tc.tile_pool(name="w", bufs=1))
        qkv = ctx.enter_context(tc.tile_pool(name="qkv", bufs=1))
        xpool = ctx.enter_context(tc.tile_pool(name="xp", bufs=2))
        probs_pool = ctx.enter_context(tc.tile_pool(name="probs", bufs=4))
        attn_pool = ctx.enter_context(tc.tile_pool(name="attn", bufs=9))
        rfpool = ctx.enter_context(tc.tile_pool(name="rf", bufs=3))
        rbpool = ctx.enter_context(tc.tile_pool(name="rb", bufs=12))
        aupool = ctx.enter_context(tc.tile_pool(name="aupool", bufs=11))
        outst = ctx.enter_context(tc.tile_pool(name="outst", bufs=2))
        # PSUM: sc 2x2 banks + pv 2x1 + o (proj/outproj) 2x1 = 8 banks
        sc_ps = ctx.enter_context(
            tc.tile_pool(name="sc_ps", bufs=2, space="PSUM"))
        pv_ps = ctx.enter_context(
            tc.tile_pool(name="pv_ps", bufs=2, space="PSUM"))
        o_ps = ctx.enter_context(
            tc.tile_pool(name="o_ps", bufs=2, space="PSUM"))

        # first V block + wv first (they gate the first matmul); consts and
        # the other weights go behind them on their queues
        xv_blk0 = xpool.tile([P, E_CH, QB], DT, tag="xv", name="xvblk")
        nc.sync.dma_start(xv_blk0, xv_t[0])
        w_tiles = {}
        w_tiles["wv"] = wpool.tile([P, E_CH, DH], DT, tag="wv", name="wv")
        nc.scalar.dma_start(w_tiles["wv"], wv_t)

        bq_sb = consts.tile([P, M_CH], F32)
        nc.sync.dma_start(bq_sb, bq_in)
        bk_sb = consts.tile([P, M_CH], F32)
        nc.sync.dma_start(bk_sb, bk_in)
        bv_sb = consts.tile([P, DH], F32)
        nc.sync.dma_start(bv_sb, bv_in)
        tri_sb = consts.tile([P, KC], DT)
        nc.sync.dma_start(tri_sb, dmask_in)
        ones65 = consts.tile([65, 64], DT)
        nc.vector.memset(ones65[64:65, :], 1.0)

        for name in ("wk", "wq"):
            w_tiles[name] = wpool.tile([P, E_CH, DH], DT, tag=name,
                                       name=name)
            nc.scalar.dma_start(w_tiles[name], {"wk": wk_t,
                                                "wq": wq_t}[name])
        wo_sb = wpool.tile([P, M_CH, D], DT, tag="wo")

        qT = qkv.tile([P, M_CH, S], DT, tag="qT")
        kT = qkv.tile([P, M_CH, S], DT, tag="kT")
        v_aug = qkv.tile([P, N_KC, NH_G, 65], DT, tag="v_aug")
        if use_bf16:
            nc.gpsimd.memset(v_aug[:, :, :, 64:65], 1.0)
        else:
            nc.gpsimd.dma_start(v_aug[:, :, :, 64:65], ones_v_in)

        # ---------------- attention building blocks ----------------
        def attn_m_group(qb, m, mq_work, filler=None):
            """scores+exp+PV+drain for one (qb, m); yields once per chunk
            so the caller can interleave other work into the PE queue.
            `filler` (if given) is invoked every 2nd chunk to splice ~1us
            of independent PE work into the queue without starving exp."""
            n_kc = (qb + 1) * R if causal else N_KC
            pv_t = [pv_ps.tile([65, QB], F32, tag="pv", name=f"pv{hl}")
                    for hl in (0, 1)]

            def emit_pv(kc, pt, c0):
                for hl in (0, 1):
                    nc.tensor.matmul(
                        pv_t[hl][:, c0:QB],
                        lhsT=v_aug[:, kc, 2 * m + hl, :],
                        rhs=pt[:, hl, c0:],
                        start=(kc == 0), stop=(kc == n_kc - 1),
                    )

            prev = None
            for kc in range(n_kc):
                r = kc - (n_kc - R) if causal else -1
                c0 = KC * r if r > 0 else 0
                sc2 = sc_ps.tile([P, 2, QB], F32, tag="sc", name="sc2")
                for hl in (0, 1):
                    rows = slice(64 * hl, 64 * hl + 64)
                    nc.tensor.matmul(
                        sc2[:, hl, c0:QB],
                        lhsT=kT[rows, m, kc * KC:(kc + 1) * KC],
                        rhs=qT[rows, m, qb * QB + c0:(qb + 1) * QB],
                        start=True, stop=True,
                    )
                pt = probs_pool.tile([P, 2, QB], DT, tag="pt")
                nc.scalar.activation(pt[:, :, c0:], sc2[:, :, c0:QB],
                                     AF.Exp, scale=float(SCALE))
                if r >= 0:
                    for hl in (0, 1):
                        nc.gpsimd.tensor_tensor(
                            pt[:, hl, c0:c0 + KC], pt[:, hl, c0:c0 + KC],
                            tri_sb, ALU.mult)
                if prev is not None:
                    emit_pv(*prev)
                prev = (kc, pt, c0)
                if filler is not None and kc % 2 == 1:
                    filler()
                yield
            emit_pv(*prev)
            # stage both PV tiles (attn rows + sums row) to SBUF in f32
            # first so the PSUM slots free after just two DVE copies
            attn_us = []
            for hl in (0, 1):
                au65 = aupool.tile([65, QB], F32, tag="attn_u",
                                   name=f"au65_{hl}")
                nc.vector.tensor_copy(au65, pv_t[hl][0:65, 0:QB])
                attn_us.append(au65)
            rcps = []
            for hl in (0, 1):
                # reciprocal over all 65 rows: the custom-DVE op
                # mis-addresses at non-zero base partitions on HW, and DVE
                # cost scales with free size only. Rows 0-63 are junk;
                # only row 64 (the sums row) is consumed.
                rcp_f = rfpool.tile([65, QB], F32, tag="rcp_f", name="rcp_f")
                nc.vector.reciprocal_approx_fast(rcp_f, attn_us[hl])
                rcpb = rbpool.tile([65, QB], DT, tag="rcpb", name="rcpb")
                nc.vector.tensor_copy(rcpb[64:65, :], rcp_f[64:65, :])
                rcps.append(rcpb)
            mq_work.append((m, attn_us, rcps))
            yield

        class QbState:
            def __init__(s, qb):
                s.qb = qb
                s.mq = []
                s.attn_tiles = []

        def norm_gen(st):
            """Per-m normalize: K=1 matmul broadcasts the recip row from
            partition 64 to 64 partitions, then a DVE multiply."""
            for m, attn_us, rcps in st.mq:
                attn_m = attn_pool.tile([P, QB], DT, tag="attn",
                                        name="attn_m")
                for hl in (0, 1):
                    rb = o_ps.tile([64, QB], F32, tag="o", name="rb")
                    nc.tensor.matmul(rb, lhsT=ones65[64:65, :],
                                     rhs=rcps[hl][64:65, :],
                                     start=True, stop=True)
                    nc.vector.tensor_tensor(
                        attn_m[64 * hl:64 * hl + 64, :],
                        attn_us[hl][0:64, :], rb, ALU.mult)
                st.attn_tiles.append(attn_m)
                yield

        def outproj_gen(st):
            for ssub in range(QB // P):
                stt = outst.tile([P, O_N, 512], F32, tag="st", name="stt")
                for nout in range(O_N):
                    pso = o_ps.tile([P, 512], F32, tag="o", name="pso")
                    for m in range(M_CH):
                        nc.tensor.matmul(
                            pso,
                            lhsT=st.attn_tiles[m][:, ssub * P:(ssub + 1) * P],
                            rhs=wo_sb[:, m, nout * 512:(nout + 1) * 512],
                            start=(m == 0), stop=(m == M_CH - 1),
                        )
                    nc.vector.tensor_copy(stt[:, nout], pso)
                    yield
                r0 = st.qb * QB + ssub * P
                nc.sync.dma_start(out_p[r0:r0 + P, :],
                                  stt.rearrange("p a b -> p (a b)"))

        # ---------------- projections ----------------
        def v_block(n, do_pull):
            if n == 0:
                xblk = xv_blk0
            else:
                xblk = xpool.tile([P, E_CH, QB], DT, tag="xv", name="xvblk")
                nc.sync.dma_start(xblk, xv_t[n])
            for sc in range(QB // P):
                ps = o_ps.tile([P, DH], F32, tag="o", name="psv")
                for e in range(E_CH):
                    nc.tensor.matmul(
                        ps,
                        lhsT=xblk[:, e, sc * P:(sc + 1) * P],
                        rhs=w_tiles["wv"][:, e, :],
                        start=(e == 0), stop=(e == E_CH - 1),
                    )
                kc = n * (QB // P) + sc
                nc.vector.tensor_tensor(
                    v_aug[:, kc, :, 0:64],
                    ps.rearrange("p (h e) -> p h e", h=NH_G),
                    bv_sb.rearrange("p (h e) -> p h e", h=NH_G),
                    ALU.add,
                )
                if do_pull:
                    next_filler()

        def kq_load(phase, n2):
            x_in = xk_t if phase == "k" else xq_t
            xblk = xpool.tile([P, E_CH, WB], DT, tag="x" + phase,
                              name="xblk")
            nc.sync.dma_start(xblk, x_in[n2])
            return xblk

        def kq_group(phase, n2, m, xblk):
            w_sb = w_tiles["wk" if phase == "k" else "wq"]
            b_sb = bk_sb if phase == "k" else bq_sb
            ps = o_ps.tile([P, WB], F32, tag="o", name="pskq")
            for e in range(E_CH):
                nc.tensor.matmul(
                    ps,
                    lhsT=w_sb[:, e, m * P:(m + 1) * P],
                    rhs=xblk[:, e, :],
                    start=(e == 0), stop=(e == E_CH - 1),
                )
            dstp = kT if phase == "k" else qT
            nc.vector.tensor_scalar_add(
                dstp[:, m, n2 * WB:(n2 + 1) * WB], ps, b_sb[:, m:m + 1])

        # prologue attention generators for qb0 (and qb1 if present)
        states = {0: QbState(0)}

        def gen_qb(qb):
            for m in range(M_CH):
                yield from attn_m_group(qb, m, states[qb].mq)
        genA = gen_qb(0)
        if N_QB > 1:
            states[1] = QbState(1)
            genB = gen_qb(1)
        else:
            genB = iter(())

        def pull(g):
            try:
                next(g)
                return True
            except StopIteration:
                return False

        def next_filler():
            if not pull(genA):
                pull(genB)

        # V0, K0 straight through; attention pulls start right after the
        # first Q0 m-group (qb0 m-group m needs only kT/qT block-0 group m
        # and v_aug block 0). Blocks >= 2 are DEFERRED into the attention
        # windows (the attention phase is exp-bound, the projection phase
        # PE-bound — deferral fills the attention phase's idle PE).
        v_block(0, do_pull=False)
        xb = kq_load("k", 0)
        for m in range(M_CH):
            kq_group("k", 0, m, xb)
        xb = kq_load("q", 0)
        for m in range(M_CH):
            kq_group("q", 0, m, xb)
            next_filler()
        n_early = min(2, NW)
        for n2 in range(1, n_early):
            v_block(n2, do_pull=True)
            for phase in ("k", "q"):
                xb = kq_load(phase, n2)
                for m in range(M_CH):
                    kq_group(phase, n2, m, xb)
                    next_filler()
        nc.scalar.dma_start(wo_sb, wo_t)

        def proj_block_start(n2):
            """Eagerly issue the three x-block DMAs for a deferred block."""
            tiles = {}
            for tag, src_t in (("xv", xv_t), ("xk", xk_t), ("xq", xq_t)):
                xblk = xpool.tile([P, E_CH, QB], DT, tag=tag, name="xvblk")
                nc.sync.dma_start(xblk, src_t[n2])
                tiles[tag] = xblk
            return tiles

        def proj_block_gen(n2, tiles):
            """Deferred projection block as a fine-grained generator:
            yields every half accumulation group (~4 matmuls)."""
            xblk = tiles["xv"]
            for sc in range(QB // P):
                ps = o_ps.tile([P, DH], F32, tag="o", name="psv")
                for e in range(E_CH):
                    nc.tensor.matmul(
                        ps,
                        lhsT=xblk[:, e, sc * P:(sc + 1) * P],
                        rhs=w_tiles["wv"][:, e, :],
                        start=(e == 0), stop=(e == E_CH - 1),
                    )
                    if e == E_CH // 2 - 1:
                        yield
                kc = n2 * (QB // P) + sc
                nc.vector.tensor_tensor(
                    v_aug[:, kc, :, 0:64],
                    ps.rearrange("p (h e) -> p h e", h=NH_G),
                    bv_sb.rearrange("p (h e) -> p h e", h=NH_G),
                    ALU.add,
                )
                yield
            for phase in ("k", "q"):
                xb = tiles["x" + phase]
                for m in range(M_CH):
                    w_sb = w_tiles["wk" if phase == "k" else "wq"]
                    b_sb = bk_sb if phase == "k" else bq_sb
                    ps = o_ps.tile([P, WB], F32, tag="o", name="pskq")
                    for e in range(E_CH):
                        nc.tensor.matmul(
                            ps,
                            lhsT=w_sb[:, e, m * P:(m + 1) * P],
                            rhs=xb[:, e, :],
                            start=(e == 0), stop=(e == E_CH - 1),
                        )
                        if e == E_CH // 2 - 1:
                            yield
                    dstp = kT if phase == "k" else qT
                    nc.vector.tensor_scalar_add(
                        dstp[:, m, n2 * WB:(n2 + 1) * WB], ps,
                        b_sb[:, m:m + 1])
                    yield

        # fill queue: pending fine-grained PE work (norm / outproj /
        # deferred projection blocks), spliced between attention chunks
        from collections import deque
        from itertools import chain

        fill_q = deque()

        def fill():
            while fill_q:
                if pull(fill_q[0]):
                    return True
                fill_q.popleft()
            return False

        def drain(gen):
            """Emit everything in fill_q up to and including `gen`."""
            while fill_q and fill_q[0] is not gen:
                while pull(fill_q[0]):
                    pass
                fill_q.popleft()
            if fill_q and fill_q[0] is gen:
                while pull(gen):
                    pass
                fill_q.popleft()

        # finish qb0 attention, then interleave qb0 norm+outproj and the
        # block-2 projections with the qb1 attention drain
        for _ in genA:
            pass
        fill_q.append(chain(norm_gen(states[0]), outproj_gen(states[0])))
        blk2 = (proj_block_gen(2, proj_block_start(2)) if NW > 2
                else iter(()))
        fill_q.append(blk2)
        i = 0
        while pull(genB):
            i += 1
            if i % 2 == 0:
                fill()
        drain(blk2)

        # ---------------- steady state: qb = 2.. ----------------
        # per qb: attn(qb) chunks interleaved (via the in-group filler)
        # with the previous qb's norm+outproj and the next deferred
        # projection block; this qb's norm runs one m-group delayed
        seqN_prev = norm_gen(states[1]) if N_QB > 1 else None
        for qb in range(2, N_QB):
            states[qb] = QbState(qb)
            if seqN_prev is not None:
                fill_q.append(seqN_prev)
            fill_q.append(outproj_gen(states[qb - 1]))
            blk = (proj_block_gen(qb + 1, proj_block_start(qb + 1))
                   if qb + 1 < NW else iter(()))
            fill_q.append(blk)
            seqN = norm_gen(states[qb])
            for m in range(M_CH):
                for _ in attn_m_group(qb, m, states[qb].mq, filler=fill):
                    pass
                if m >= 1:
                    pull(seqN)
                fill()
            drain(blk)
            seqN_prev = seqN

        # tail: last qb's remaining norm + outproj
        while fill():
            pass
        last = states[N_QB - 1]
        tail_norm = seqN_prev if seqN_prev is not None else norm_gen(last)
        for _ in tail_norm:
            pass
        for _ in outproj_gen(last):
            pass
    nc.compile()
    return nc


def make_consts(S, use_bf16):
    """Host-built 0/1 upper-triangular (key<=query) mask for diag chunks."""
    npdt = _np_dt(use_bf16)
    i = np.arange(P)[:, None]
    j = np.arange(KC)[None, :]
    return (i <= j).astype(npdt)


def core_inputs(Q, K, V, W_q, b_q, W_k, b_k, W_v, b_v, W_o, b, hg, S, use_bf16):
    """Build the per-core input map (host-side slicing/transposition/casts)."""
    npdt = _np_dt(use_bf16)
    d = _dims(S)
    M_CH = d["M_CH"]
    rows = slice(hg * DH, (hg + 1) * DH)

    QB = d["QB"]
    E_CH = D // P

    def xt(x):
        # [S, D] -> [N_QB, P, E_CH, QB]: per-core x, transposed and tiled so
        # each device block load is a contiguous DMA.
        a = np.asarray(x, np.float32).T.astype(npdt)      # [D, S]
        a = a.reshape(E_CH, P, S // QB, QB).transpose(2, 1, 0, 3)
        return np.ascontiguousarray(a)

    def wt(w):
        # [DH, D] slice -> W^T tiled [P, E_CH, DH]
        a = np.asarray(w, np.float32).T.astype(npdt)      # [D, DH]
        return np.ascontiguousarray(
            a.reshape(E_CH, P, DH).transpose(1, 0, 2))

    a_wo = np.asarray(W_o[:, rows], np.float32).T.astype(npdt)  # [DH, D]
    wo_prep = np.ascontiguousarray(
        a_wo.reshape(M_CH, P, D).transpose(1, 0, 2))

    dmask = make_consts(S, use_bf16)
    return {
        "xq_t": xt(Q[b]), "xk_t": xt(K[b]), "xv_t": xt(V[b]),
        "wq_t": wt(W_q[rows]), "wk_t": wt(W_k[rows]), "wv_t": wt(W_v[rows]),
        "wo_t": wo_prep,
        "bq_p": np.ascontiguousarray(
            np.asarray(b_q[rows], np.float32).reshape(M_CH, P).T),
        "bk_p": np.ascontiguousarray(
            np.asarray(b_k[rows], np.float32).reshape(M_CH, P).T),
        "bv_r": np.broadcast_to(
            np.asarray(b_v[rows], np.float32), (P, DH)).copy(),
        "dmask": dmask,
        "ones_v": np.ones((P, d["N_KC"], NH_G, 1), npdt),
    }


def _np_reference(Q, K, V, mask, W_q, b_q, W_k, b_k, W_v, b_v, W_o, b_o):
    """Exact numpy fallback for arbitrary masks."""
    q = (Q @ W_q.T + b_q).reshape(B, S_FULL, H, DK).transpose(0, 2, 1, 3)
    k = (K @ W_k.T + b_k).reshape(B, S_FULL, H, DK).transpose(0, 2, 1, 3)
    v = (V @ W_v.T + b_v).reshape(B, S_FULL, H, DK).transpose(0, 2, 1, 3)
    scores = np.einsum("bhqd,bhkd->bhqk", q, k) / np.sqrt(np.float32(DK))
    scores = np.where(mask == 0, np.finfo(np.float32).min, scores)
    scores -= scores.max(-1, keepdims=True)
    probs = np.exp(scores)
    probs /= probs.sum(-1, keepdims=True)
    out = np.einsum("bhqk,bhkd->bhqd", probs, v)
    out = out.transpose(0, 2, 1, 3).reshape(B, S_FULL, D)
    return (out @ W_o.T + b_o).astype(np.float32)


def kernel(Q, K, V, mask, W_q, b_q, W_k, b_k, W_v, b_v, W_o, b_o):
    Q = np.asarray(Q, np.float32)
    K = np.asarray(K, np.float32)
    V = np.asarray(V, np.float32)
    mask = np.asarray(mask)

    m2 = mask.reshape(mask.shape[-2], mask.shape[-1])
    if np.array_equal(m2 != 0, np.tril(np.ones(m2.shape, bool))):
        causal = True
    elif (m2 != 0).all():
        causal = False
    else:
        return _np_reference(Q, K, V, mask, W_q, b_q, W_k, b_k, W_v, b_v,
                             W_o, b_o)

    use_bf16 = os.environ.get("MHA_KERNEL_DTYPE", "bf16") == "bf16"
    from concourse.bass_utils import run_bass_kernel_spmd

    key = (causal, S_FULL, use_bf16)
    if key not in _PROG_CACHE:
        _PROG_CACHE[key] = build_program(causal, S_FULL, use_bf16)
    nc = _PROG_CACHE[key]

    in_maps = []
    for c in range(8):
        b, hg = divmod(c, 2)
        in_maps.append(core_inputs(Q, K, V, W_q, b_q, W_k, b_k, W_v, b_v,
                                   W_o, b, hg, S_FULL, use_bf16))

    trace = os.environ.get("MHA_KERNEL_TRACE", "0") == "1"
    kw = {}
    if trace:
        kw = {"trace": True,
              "trace_cores": [int(x) for x in os.environ.get(
                  "MHA_TRACE_CORES", "0").split(",")]}
    n_cores = int(os.environ.get("MHA_CORES", "8"))
    res = run_bass_kernel_spmd(nc, in_maps[:n_cores],
                               core_ids=list(range(n_cores)), **kw)
    kernel.last_results = res

    b_o32 = np.asarray(b_o, np.float32)
    out = np.zeros((B, S_FULL, D), np.float32)
    for b in range(B):
        if 2 * b + 1 < n_cores:
            out[b] = (res.results[2 * b]["out_p"]
                      + res.results[2 * b + 1]["out_p"] + b_o32[None, :])
    return out


kernel.last_results = None


# revision 26
# speedup vs baseline: 1.0398x; 1.0398x over previous
"""MultiHeadAttention Trainium2 kernel (8 NeuronCores).

Sharding: core c handles batch b = c // 2 and head-group hg = c % 2
(8 of 16 heads, 512 of 1024 model dims). Attention is embarrassingly
parallel over (b, hg); the output projection is computed per head-group
against the matching W_o columns, yielding partial outputs that the host
sums (plus b_o).

Device dataflow (per core), all in "transposed" layouts so no on-device
transposes are ever needed:
  qT = Wq_hg @ Xq^T      [dh=512, S]   (lhsT = Wq_hg^T, rhs = Xq^T)
  kT = Wk_hg @ Xk^T      [dh=512, S]
  v  = Xv @ Wv_hg^T      [S, dh=512]   (+ ones column per head for sums)
  scores_T[k, q]: per head-pair (m) and half (hl), keys on partitions;
    the two hl matmuls run CONCURRENTLY as 64-row PE tiles (row_grp h0 /
    h64, auto-derived from lhsT base partitions) and write one 2-bank
    PSUM tile [128, 2, QB] so a single fused ACT exp covers the pair.
    Diagonal chunks restrict columns to the causally-reachable range;
    the 128-col triangular block is zeroed after exp with one small
    Pool-engine multiply.
  probs -> PV: attn_T[d, q] + sums row accumulated in PSUM [65, QB],
    rhs column-restricted per chunk.
  normalize: per head-pair reciprocal_approx_fast directly on the PSUM
    sums row (partition 64), cast to bf16, then a K=1 ones-matmul
    broadcast to 64 partitions and a DVE multiply — no DMA round-trips,
    no batched-recip barrier (norm pipelines per m-group).
  out_partial = attn^T-matmul with Wo columns.

Schedule (emission order == engine-queue order):
  V proj | K0 Q0 | K1..K3 Q1..Q3 interleaved with qb0+qb1 attention
  (generators yield per key-chunk so exp/PV ride the projection window)
  | qb0 norm+outproj interleaved with the qb1 attention drain
  | for qb in 2..: attn(qb) m-groups interleaved with norm+outproj(qb-1)
  | norm+outproj(last).
"""

import os

import numpy as np

B, S_FULL, D = 4, 2048, 1024
H, DK = 16, 64
NH_G = 8          # heads per core
DH = NH_G * DK    # 512 dims per core
P = 128
KC = 128          # key chunk (PE contraction)
SCALE = 1.0 / np.sqrt(np.float32(DK))

_PROG_CACHE = {}


def _dims(S):
    QB = min(512, S)
    return {
        "S": S, "QB": QB, "N_QB": S // QB, "N_KC": S // KC,
        "R": QB // KC, "E_CH": D // P, "M_CH": DH // P, "O_N": D // 512,
    }


def _np_dt(use_bf16):
    if use_bf16:
        import ml_dtypes
        return ml_dtypes.bfloat16
    return np.float32


def build_program(causal, S, use_bf16=True):
    """Build the single-core Bass/Tile program (same program on all 8 cores)."""
    from contextlib import ExitStack

    import concourse.bass as bass
    import concourse.tile as tile
    from concourse import bacc, mybir

    d = _dims(S)
    QB, N_QB, N_KC, R, E_CH, M_CH, O_N = (
        d["QB"], d["N_QB"], d["N_KC"], d["R"], d["E_CH"], d["M_CH"], d["O_N"])

    DT = mybir.dt.bfloat16 if use_bf16 else mybir.dt.float32r
    F32 = mybir.dt.float32
    AF = mybir.ActivationFunctionType
    ALU = mybir.AluOpType

    WB = QB
    NW = S // WB

    nc = bacc.Bacc("TRN2", target_bir_lowering=False, debug=False)

    NB = S // QB
    xq_t = nc.dram_tensor("xq_t", [NB, P, E_CH, QB], DT,
                          kind="ExternalInput").ap()
    xk_t = nc.dram_tensor("xk_t", [NB, P, E_CH, QB], DT,
                          kind="ExternalInput").ap()
    xv_t = nc.dram_tensor("xv_t", [NB, P, E_CH, QB], DT,
                          kind="ExternalInput").ap()
    wq_t = nc.dram_tensor("wq_t", [P, E_CH, DH], DT,
                          kind="ExternalInput").ap()
    wk_t = nc.dram_tensor("wk_t", [P, E_CH, DH], DT,
                          kind="ExternalInput").ap()
    wv_t = nc.dram_tensor("wv_t", [P, E_CH, DH], DT,
                          kind="ExternalInput").ap()
    wo_t = nc.dram_tensor("wo_t", [P, M_CH, D], DT,
                          kind="ExternalInput").ap()
    bq_in = nc.dram_tensor("bq_p", [P, M_CH], F32, kind="ExternalInput").ap()
    bk_in = nc.dram_tensor("bk_p", [P, M_CH], F32, kind="ExternalInput").ap()
    bv_in = nc.dram_tensor("bv_r", [P, DH], F32, kind="ExternalInput").ap()
    dmask_in = nc.dram_tensor("dmask", [P, KC], DT,
                              kind="ExternalInput").ap()
    ones_v_in = nc.dram_tensor("ones_v", [P, N_KC, NH_G, 1], DT,
                               kind="ExternalInput").ap()
    out_p = nc.dram_tensor("out_p", [S, D], F32, kind="ExternalOutput").ap()

    with tile.TileContext(nc) as tc, ExitStack() as ctx:
        consts = ctx.enter_context(tc.tile_pool(name="consts", bufs=1))
        wpool = ctx.enter_context(tc.tile_pool(name="w", bufs=1))
        qkv = ctx.enter_context(tc.tile_pool(name="qkv", bufs=1))
        xpool = ctx.enter_context(tc.tile_pool(name="xp", bufs=2))
        probs_pool = ctx.enter_context(tc.tile_pool(name="probs", bufs=4))
        attn_pool = ctx.enter_context(tc.tile_pool(name="attn", bufs=9))
        rfpool = ctx.enter_context(tc.tile_pool(name="rf", bufs=3))
        rbpool = ctx.enter_context(tc.tile_pool(name="rb", bufs=12))
        aupool = ctx.enter_context(tc.tile_pool(name="aupool", bufs=11))
        outst = ctx.enter_context(tc.tile_pool(name="outst", bufs=2))
        # PSUM: sc 2x2 banks + pv 2x1 + o (proj/outproj) 2x1 = 8 banks
        sc_ps = ctx.enter_context(
            tc.tile_pool(name="sc_ps", bufs=2, space="PSUM"))
        pv_ps = ctx.enter_context(
            tc.tile_pool(name="pv_ps", bufs=2, space="PSUM"))
        o_ps = ctx.enter_context(
            tc.tile_pool(name="o_ps", bufs=2, space="PSUM"))

        # first V block + wv first (they gate the first matmul); consts and
        # the other weights go behind them on their queues
        xv_blk0 = xpool.tile([P, E_CH, QB], DT, tag="xv", name="xvblk")
        nc.sync.dma_start(xv_blk0, xv_t[0])
        w_tiles = {}
        w_tiles["wv"] = wpool.tile([P, E_CH, DH], DT, tag="wv", name="wv")
        nc.scalar.dma_start(w_tiles["wv"], wv_t)

        bq_sb = consts.tile([P, M_CH], F32)
        nc.sync.dma_start(bq_sb, bq_in)
        bk_sb = consts.tile([P, M_CH], F32)
        nc.sync.dma_start(bk_sb, bk_in)
        bv_sb = consts.tile([P, DH], F32)
        nc.sync.dma_start(bv_sb, bv_in)
        tri_sb = consts.tile([P, KC], DT)
        nc.sync.dma_start(tri_sb, dmask_in)
        ones65 = consts.tile([65, 64], DT)
        nc.vector.memset(ones65[64:65, :], 1.0)

        for name in ("wk", "wq"):
            w_tiles[name] = wpool.tile([P, E_CH, DH], DT, tag=name,
                                       name=name)
            nc.scalar.dma_start(w_tiles[name], {"wk": wk_t,
                                                "wq": wq_t}[name])
        wo_sb = wpool.tile([P, M_CH, D], DT, tag="wo")

        qT = qkv.tile([P, M_CH, S], DT, tag="qT")
        kT = qkv.tile([P, M_CH, S], DT, tag="kT")
        v_aug = qkv.tile([P, N_KC, NH_G, 65], DT, tag="v_aug")
        if use_bf16:
            nc.gpsimd.memset(v_aug[:, :, :, 64:65], 1.0)
        else:
            nc.gpsimd.dma_start(v_aug[:, :, :, 64:65], ones_v_in)

        # ---------------- attention building blocks ----------------
        def attn_m_group(qb, m, mq_work, filler=None):
            """scores+exp+PV+drain for one (qb, m); yields once per chunk
            so the caller can interleave other work into the PE queue.
            `filler` (if given) is invoked every 2nd chunk to splice ~1us
            of independent PE work into the queue without starving exp."""
            n_kc = (qb + 1) * R if causal else N_KC
            pv_t = [pv_ps.tile([65, QB], F32, tag="pv", name=f"pv{hl}")
                    for hl in (0, 1)]

            def emit_pv(kc, pt, c0):
                for hl in (0, 1):
                    nc.tensor.matmul(
                        pv_t[hl][:, c0:QB],
                        lhsT=v_aug[:, kc, 2 * m + hl, :],
                        rhs=pt[:, hl, c0:],
                        start=(kc == 0), stop=(kc == n_kc - 1),
                    )

            prev = None
            for kc in range(n_kc):
                r = kc - (n_kc - R) if causal else -1
                c0 = KC * r if r > 0 else 0
                sc2 = sc_ps.tile([P, 2, QB], F32, tag="sc", name="sc2")
                for hl in (0, 1):
                    rows = slice(64 * hl, 64 * hl + 64)
                    nc.tensor.matmul(
                        sc2[:, hl, c0:QB],
                        lhsT=kT[rows, m, kc * KC:(kc + 1) * KC],
                        rhs=qT[rows, m, qb * QB + c0:(qb + 1) * QB],
                        start=True, stop=True,
                    )
                pt = probs_pool.tile([P, 2, QB], DT, tag="pt")
                nc.scalar.activation(pt[:, :, c0:], sc2[:, :, c0:QB],
                                     AF.Exp, scale=float(SCALE))
                if r >= 0:
                    for hl in (0, 1):
                        nc.gpsimd.tensor_tensor(
                            pt[:, hl, c0:c0 + KC], pt[:, hl, c0:c0 + KC],
                            tri_sb, ALU.mult)
                if prev is not None:
                    emit_pv(*prev)
                prev = (kc, pt, c0)
                if filler is not None and kc % 3 == 2:
                    filler()
                yield
            emit_pv(*prev)
            # stage both PV tiles (attn rows + sums row) to SBUF in f32
            # first so the PSUM slots free after just two DVE copies
            attn_us = []
            for hl in (0, 1):
                au65 = aupool.tile([65, QB], F32, tag="attn_u",
                                   name=f"au65_{hl}")
                nc.vector.tensor_copy(au65, pv_t[hl][0:65, 0:QB])
                attn_us.append(au65)
            rcps = []
            for hl in (0, 1):
                # reciprocal over all 65 rows: the custom-DVE op
                # mis-addresses at non-zero base partitions on HW, and DVE
                # cost scales with free size only. Rows 0-63 are junk;
                # only row 64 (the sums row) is consumed.
                rcp_f = rfpool.tile([65, QB], F32, tag="rcp_f", name="rcp_f")
                nc.vector.reciprocal_approx_fast(rcp_f, attn_us[hl])
                rcpb = rbpool.tile([65, QB], DT, tag="rcpb", name="rcpb")
                nc.vector.tensor_copy(rcpb[64:65, :], rcp_f[64:65, :])
                rcps.append(rcpb)
            mq_work.append((m, attn_us, rcps))
            yield

        class QbState:
            def __init__(s, qb):
                s.qb = qb
                s.mq = []
                s.attn_tiles = []

        def norm_gen(st):
            """Per-m normalize: K=1 matmul broadcasts the recip row from
            partition 64 to 64 partitions, then a DVE multiply."""
            for m, attn_us, rcps in st.mq:
                attn_m = attn_pool.tile([P, QB], DT, tag="attn",
                                        name="attn_m")
                for hl in (0, 1):
                    rb = o_ps.tile([64, QB], F32, tag="o", name="rb")
                    nc.tensor.matmul(rb, lhsT=ones65[64:65, :],
                                     rhs=rcps[hl][64:65, :],
                                     start=True, stop=True)
                    nc.vector.tensor_tensor(
                        attn_m[64 * hl:64 * hl + 64, :],
                        attn_us[hl][0:64, :], rb, ALU.mult)
                st.attn_tiles.append(attn_m)
                yield

        def outproj_gen(st):
            for ssub in range(QB // P):
                stt = outst.tile([P, O_N, 512], F32, tag="st", name="stt")
                for nout in range(O_N):
                    pso = o_ps.tile([P, 512], F32, tag="o", name="pso")
                    for m in range(M_CH):
                        nc.tensor.matmul(
                            pso,
                            lhsT=st.attn_tiles[m][:, ssub * P:(ssub + 1) * P],
                            rhs=wo_sb[:, m, nout * 512:(nout + 1) * 512],
                            start=(m == 0), stop=(m == M_CH - 1),
                        )
                    nc.vector.tensor_copy(stt[:, nout], pso)
                    yield
                r0 = st.qb * QB + ssub * P
                nc.sync.dma_start(out_p[r0:r0 + P, :],
                                  stt.rearrange("p a b -> p (a b)"))

        # ---------------- projections ----------------
        def v_block(n, do_pull):
            if n == 0:
                xblk = xv_blk0
            else:
                xblk = xpool.tile([P, E_CH, QB], DT, tag="xv", name="xvblk")
                nc.sync.dma_start(xblk, xv_t[n])
            for sc in range(QB // P):
                ps = o_ps.tile([P, DH], F32, tag="o", name="psv")
                for e in range(E_CH):
                    nc.tensor.matmul(
                        ps,
                        lhsT=xblk[:, e, sc * P:(sc + 1) * P],
                        rhs=w_tiles["wv"][:, e, :],
                        start=(e == 0), stop=(e == E_CH - 1),
                    )
                kc = n * (QB // P) + sc
                nc.vector.tensor_tensor(
                    v_aug[:, kc, :, 0:64],
                    ps.rearrange("p (h e) -> p h e", h=NH_G),
                    bv_sb.rearrange("p (h e) -> p h e", h=NH_G),
                    ALU.add,
                )
                if do_pull:
                    next_filler()

        def kq_load(phase, n2):
            x_in = xk_t if phase == "k" else xq_t
            xblk = xpool.tile([P, E_CH, WB], DT, tag="x" + phase,
                              name="xblk")
            nc.sync.dma_start(xblk, x_in[n2])
            return xblk

        def kq_group(phase, n2, m, xblk):
            w_sb = w_tiles["wk" if phase == "k" else "wq"]
            b_sb = bk_sb if phase == "k" else bq_sb
            ps = o_ps.tile([P, WB], F32, tag="o", name="pskq")
            for e in range(E_CH):
                nc.tensor.matmul(
                    ps,
                    lhsT=w_sb[:, e, m * P:(m + 1) * P],
                    rhs=xblk[:, e, :],
                    start=(e == 0), stop=(e == E_CH - 1),
                )
            dstp = kT if phase == "k" else qT
            nc.vector.tensor_scalar_add(
                dstp[:, m, n2 * WB:(n2 + 1) * WB], ps, b_sb[:, m:m + 1])

        # prologue attention generators for qb0 (and qb1 if present)
        states = {0: QbState(0)}

        def gen_qb(qb):
            for m in range(M_CH):
                yield from attn_m_group(qb, m, states[qb].mq)
        genA = gen_qb(0)
        if N_QB > 1:
            states[1] = QbState(1)
            genB = gen_qb(1)
        else:
            genB = iter(())

        def pull(g):
            try:
                next(g)
                return True
            except StopIteration:
                return False

        def next_filler():
            if not pull(genA):
                pull(genB)

        # V0, K0 straight through; attention pulls start right after the
        # first Q0 m-group (qb0 m-group m needs only kT/qT block-0 group m
        # and v_aug block 0). Blocks >= 2 are DEFERRED into the attention
        # windows (the attention phase is exp-bound, the projection phase
        # PE-bound — deferral fills the attention phase's idle PE).
        v_block(0, do_pull=False)
        xb = kq_load("k", 0)
        for m in range(M_CH):
            kq_group("k", 0, m, xb)
        xb = kq_load("q", 0)
        for m in range(M_CH):
            kq_group("q", 0, m, xb)
            next_filler()
        n_early = min(2, NW)
        for n2 in range(1, n_early):
            v_block(n2, do_pull=True)
            for phase in ("k", "q"):
                xb = kq_load(phase, n2)
                for m in range(M_CH):
                    kq_group(phase, n2, m, xb)
                    next_filler()
        nc.scalar.dma_start(wo_sb, wo_t)

        def proj_block_start(n2):
            """Eagerly issue the three x-block DMAs for a deferred block."""
            tiles = {}
            for tag, src_t in (("xv", xv_t), ("xk", xk_t), ("xq", xq_t)):
                xblk = xpool.tile([P, E_CH, QB], DT, tag=tag, name="xvblk")
                nc.sync.dma_start(xblk, src_t[n2])
                tiles[tag] = xblk
            return tiles

        def proj_block_gen(n2, tiles):
            """Deferred projection block as a fine-grained generator:
            yields every half accumulation group (~4 matmuls)."""
            xblk = tiles["xv"]
            for sc in range(QB // P):
                ps = o_ps.tile([P, DH], F32, tag="o", name="psv")
                for e in range(E_CH):
                    nc.tensor.matmul(
                        ps,
                        lhsT=xblk[:, e, sc * P:(sc + 1) * P],
                        rhs=w_tiles["wv"][:, e, :],
                        start=(e == 0), stop=(e == E_CH - 1),
                    )
                kc = n2 * (QB // P) + sc
                nc.vector.tensor_tensor(
                    v_aug[:, kc, :, 0:64],
                    ps.rearrange("p (h e) -> p h e", h=NH_G),
                    bv_sb.rearrange("p (h e) -> p h e", h=NH_G),
                    ALU.add,
                )
                yield
            for phase in ("k", "q"):
                xb = tiles["x" + phase]
                for m in range(M_CH):
                    w_sb = w_tiles["wk" if phase == "k" else "wq"]
                    b_sb = bk_sb if phase == "k" else bq_sb
                    ps = o_ps.tile([P, WB], F32, tag="o", name="pskq")
                    for e in range(E_CH):
                        nc.tensor.matmul(
                            ps,
                            lhsT=w_sb[:, e, m * P:(m + 1) * P],
                            rhs=xb[:, e, :],
                            start=(e == 0), stop=(e == E_CH - 1),
                        )
                    dstp = kT if phase == "k" else qT
                    nc.vector.tensor_scalar_add(
                        dstp[:, m, n2 * WB:(n2 + 1) * WB], ps,
                        b_sb[:, m:m + 1])
                    yield

        # fill queue: pending fine-grained PE work (norm / outproj /
        # deferred projection blocks), spliced between attention chunks
        from collections import deque
        from itertools import chain

        fill_q = deque()

        def fill():
            while fill_q:
                if pull(fill_q[0]):
                    return True
                fill_q.popleft()
            return False

        def drain(gen):
            """Emit everything in fill_q up to and including `gen`."""
            while fill_q and fill_q[0] is not gen:
                while pull(fill_q[0]):
                    pass
                fill_q.popleft()
            if fill_q and fill_q[0] is gen:
                while pull(gen):
                    pass
                fill_q.popleft()

        # finish qb0 attention, then interleave qb0 norm+outproj and the
        # block-2 projections with the qb1 attention drain
        for _ in genA:
            pass
        fill_q.append(chain(norm_gen(states[0]), outproj_gen(states[0])))
        blk2 = (proj_block_gen(2, proj_block_start(2)) if NW > 2
                else iter(()))
        fill_q.append(blk2)
        i = 0
        while pull(genB):
            i += 1
            if i % 2 == 0:
                fill()
        drain(blk2)

        # ---------------- steady state: qb = 2.. ----------------
        # per qb: attn(qb) chunks interleaved (via the in-group filler)
        # with the previous qb's norm+outproj and the next deferred
        # projection block; this qb's norm runs one m-group delayed
        seqN_prev = norm_gen(states[1]) if N_QB > 1 else None
        for qb in range(2, N_QB):
            states[qb] = QbState(qb)
            if seqN_prev is not None:
                fill_q.append(seqN_prev)
            fill_q.append(outproj_gen(states[qb - 1]))
            blk = (proj_block_gen(qb + 1, proj_block_start(qb + 1))
                   if qb + 1 < NW else iter(()))
            fill_q.append(blk)
            seqN = norm_gen(states[qb])
            for m in range(M_CH):
                for _ in attn_m_group(qb, m, states[qb].mq, filler=fill):
                    pass
                if m >= 1:
                    pull(seqN)
                fill()
            drain(blk)
            seqN_prev = seqN

        # tail: last qb's remaining norm + outproj
        while fill():
            pass
        last = states[N_QB - 1]
        tail_norm = seqN_prev if seqN_prev is not None else norm_gen(last)
        for _ in tail_norm:
            pass
        for _ in outproj_gen(last):
            pass
    nc.compile()
    return nc


def make_consts(S, use_bf16):
    """Host-built 0/1 upper-triangular (key<=query) mask for diag chunks."""
    npdt = _np_dt(use_bf16)
    i = np.arange(P)[:, None]
    j = np.arange(KC)[None, :]
    return (i <= j).astype(npdt)


def core_inputs(Q, K, V, W_q, b_q, W_k, b_k, W_v, b_v, W_o, b, hg, S, use_bf16):
    """Build the per-core input map (host-side slicing/transposition/casts)."""
    npdt = _np_dt(use_bf16)
    d = _dims(S)
    M_CH = d["M_CH"]
    rows = slice(hg * DH, (hg + 1) * DH)

    QB = d["QB"]
    E_CH = D // P

    def xt(x):
        # [S, D] -> [N_QB, P, E_CH, QB]: per-core x, transposed and tiled so
        # each device block load is a contiguous DMA.
        a = np.asarray(x, np.float32).T.astype(npdt)      # [D, S]
        a = a.reshape(E_CH, P, S // QB, QB).transpose(2, 1, 0, 3)
        return np.ascontiguousarray(a)

    def wt(w):
        # [DH, D] slice -> W^T tiled [P, E_CH, DH]
        a = np.asarray(w, np.float32).T.astype(npdt)      # [D, DH]
        return np.ascontiguousarray(
            a.reshape(E_CH, P, DH).transpose(1, 0, 2))

    a_wo = np.asarray(W_o[:, rows], np.float32).T.astype(npdt)  # [DH, D]
    wo_prep = np.ascontiguousarray(
        a_wo.reshape(M_CH, P, D).transpose(1, 0, 2))

    dmask = make_consts(S, use_bf16)
    return {
        "xq_t": xt(Q[b]), "xk_t": xt(K[b]), "xv_t": xt(V[b]),
        "wq_t": wt(W_q[rows]), "wk_t": wt(W_k[rows]), "wv_t": wt(W_v[rows]),
        "wo_t": wo_prep,
        "bq_p": np.ascontiguousarray(
            np.asarray(b_q[rows], np.float32).reshape(M_CH, P).T),
        "bk_p": np.ascontiguousarray(
            np.asarray(b_k[rows], np.float32).reshape(M_CH, P).T),
        "bv_r": np.broadcast_to(
            np.asarray(b_v[rows], np.float32), (P, DH)).copy(),
        "dmask": dmask,
        "ones_v": np.ones((P, d["N_KC"], NH_G, 1), npdt),
    }


def _np_reference(Q, K, V, mask, W_q, b_q, W_k, b_k, W_v, b_v, W_o, b_o):
    """Exact numpy fallback for arbitrary masks."""
    q = (Q @ W_q.T + b_q).reshape(B, S_FULL, H, DK).transpose(0, 2, 1, 3)
    k = (K @ W_k.T + b_k).reshape(B, S_FULL, H, DK).transpose(0, 2, 1, 3)
    v = (V @ W_v.T + b_v).reshape(B, S_FULL, H, DK).transpose(0, 2, 1, 3)
    scores = np.einsum("bhqd,bhkd->bhqk", q, k) / np.sqrt(np.float32(DK))
    scores = np.where(mask == 0, np.finfo(np.float32).min, scores)
    scores -= scores.max(-1, keepdims=True)
    probs = np.exp(scores)
    probs /= probs.sum(-1, keepdims=True)
    out = np.einsum("bhqk,bhkd->bhqd", probs, v)
    out = out.transpose(0, 2, 1, 3).reshape(B, S_FULL, D)
    return (out @ W_o.T + b_o).astype(np.float32)


def kernel(Q, K, V, mask, W_q, b_q, W_k, b_k, W_v, b_v, W_o, b_o):
    Q = np.asarray(Q, np.float32)
    K = np.asarray(K, np.float32)
    V = np.asarray(V, np.float32)
    mask = np.asarray(mask)

    m2 = mask.reshape(mask.shape[-2], mask.shape[-1])
    if np.array_equal(m2 != 0, np.tril(np.ones(m2.shape, bool))):
        causal = True
    elif (m2 != 0).all():
        causal = False
    else:
        return _np_reference(Q, K, V, mask, W_q, b_q, W_k, b_k, W_v, b_v,
                             W_o, b_o)

    use_bf16 = os.environ.get("MHA_KERNEL_DTYPE", "bf16") == "bf16"
    from concourse.bass_utils import run_bass_kernel_spmd

    key = (causal, S_FULL, use_bf16)
    if key not in _PROG_CACHE:
        _PROG_CACHE[key] = build_program(causal, S_FULL, use_bf16)
    nc = _PROG_CACHE[key]

    in_maps = []
    for c in range(8):
        b, hg = divmod(c, 2)
        in_maps.append(core_inputs(Q, K, V, W_q, b_q, W_k, b_k, W_v, b_v,
                                   W_o, b, hg, S_FULL, use_bf16))

    trace = os.environ.get("MHA_KERNEL_TRACE", "0") == "1"
    kw = {}
    if trace:
        kw = {"trace": True,
              "trace_cores": [int(x) for x in os.environ.get(
                  "MHA_TRACE_CORES", "0").split(",")]}
    n_cores = int(os.environ.get("MHA_CORES", "8"))
    res = run_bass_kernel_spmd(nc, in_maps[:n_cores],
                               core_ids=list(range(n_cores)), **kw)
    kernel.last_results = res

    b_o32 = np.asarray(b_o, np.float32)
    out = np.zeros((B, S_FULL, D), np.float32)
    for b in range(B):
        if 2 * b + 1 < n_cores:
            out[b] = (res.results[2 * b]["out_p"]
                      + res.results[2 * b + 1]["out_p"] + b_o32[None, :])
    return out


kernel.last_results = None
